# revision 12
# baseline (speedup 1.0000x reference)
"""LIAFResBlock forward on 8 Trainium2 NeuronCores (data-parallel over batch).

Self-contained: hardcodes shapes for x [16,64,8,56,56] -> out [16,128,8,28,28].

Math notes (vs the PyTorch/JAX reference):
  - conv biases are no-ops: every conv is followed by training-mode BN, which
    subtracts the per-channel mean, absorbing any per-channel constant.
  - the final mem_update on a binary {0,1} tensor is the identity because
    d = sigmoid(0.5) ~ 0.6225 and d*0.5 < 0.5, so out = lif_act(bn2(cv2)+bn_sc(sc)).
  - the first mem_update runs in "normalized" space: with a1 = g1*rstd1 (>0),
    v = m/a1 satisfies v[t] = d*v[t-1]*[v<=tau] + (cv1[t] + beta1/a1),
    spike[t] = v[t] > tau, tau = 0.5/a1. BN1 folds into a per-channel bias on
    cv1 plus a per-channel threshold.
  - BN batch stats are global over B=16: each core computes per-channel
    (sum, sumsq) partials; a tiny AllReduce combines them.
  - final threshold in raw-cv2 space: out = 1[cv2_raw > (0.5 - bsc - asc*sc
    - b2)/a2] (a2 > 0 at init since gamma2 = 1).

Implementation notes (hardware-measured):
  - conv1/shortcut run as bf16 hi+lo pairs: x = xhi + xlo and W = Whi + Wlo
    (each bf16, split host-side), rhs K=128 stacks [xhi; xlo] and each tap
    issues two 1-cycle/row streams, lhsT [Whi;Whi] then [Wlo;Wlo], giving
    (Whi+Wlo)*(xhi+xlo): bf16 products are exact in the fp32 PSUM, so this is
    an fp32 conv of ~2^-17-rounded operands (emulated: 749 output flips, rel
    1.27e-2, vs budget 2e-2). A plain fp32 conv1 costs ~2x more PE time;
    any tf32/bf16-level rounding of x explodes ~30x through the LIF
    recurrence (16k flips).
  - conv2 fp16 single-pass (spikes are exact {0,1} in fp16; w2 fp16 rounding
    costs ~590 flips).
  - zero padding is done with partial-range matmuls (each off-center tap
    writes only its valid output sub-rectangle of PSUM; the center tap goes
    first with start=True covering the full chunk), NOT with zero-ringed
    input tiles: padded tiles force 224-byte DMA descriptors (~96 GB/s
    effective, 262us for x) while unpadded tiles stream at 6.3KB/descriptor.
  - spikes are written as contiguous [128, 1568] fp16: strided elementwise
    writes on Vector/Pool run ~18ns/elem AND starve the other engine's SBUF
    access (measured 28us stretches on concurrently-running DVE ops).
  - weights are transposed host-side so every DMA is contiguous (on-device
    transposing DMA = 230k 4-byte packets = 170us).
"""
import math
import sys

import numpy as np

sys.path.insert(0, "/opt/trn_rl_repo")

import concourse.bass as bass  # noqa: E402
import concourse.bacc as bacc  # noqa: E402
import concourse.tile as tile  # noqa: E402
from concourse import mybir  # noqa: E402
from concourse.bass_utils import run_bass_kernel_spmd  # noqa: E402

dt = mybir.dt
Alu = mybir.AluOpType
Act = mybir.ActivationFunctionType

B, CIN, COUT, T, H, W = 16, 64, 128, 8, 56, 56
HO = WO = 28
NPIX = HO * WO          # 784
CHUNK = NPIX // 2       # 392 (one PSUM bank)
NCORES = 8
BPC = B // NCORES       # 2 samples per core
SPT = BPC * NPIX        # 1568 elements per fused (both-samples) timestep
NLOC = BPC * T * NPIX   # 12544 elements/channel per core
NGLOB = B * T * NPIX    # 100352 elements/channel globally
EPS = 1e-5
HW = H * W              # 3136 elements per unpadded input plane


def _ap(base, off, free):
    """Sub-view of an SBUF AP: keep partition dim, custom free dims."""
    return bass.AP(tensor=base.tensor, offset=base.offset + off,
                   ap=[base.ap[0]] + free)


def build_nc(d: float) -> bass.Bass:
    nc = bacc.Bacc("TRN2", target_bir_lowering=False, num_devices=NCORES)

    xhi_d = nc.dram_tensor("xhi", [BPC, CIN, T, H, W], dt.bfloat16,
                           kind="ExternalInput")
    xlo_d = nc.dram_tensor("xlo", [BPC, CIN, T, H, W], dt.bfloat16,
                           kind="ExternalInput")
    # host-pretransposed lhsT weights, hi/lo split, duplicated across K halves
    w1h_d = nc.dram_tensor("w1h", [2 * CIN, 9, COUT], dt.bfloat16,
                           kind="ExternalInput")   # [Whi; Whi]
    w1l_d = nc.dram_tensor("w1l", [2 * CIN, 9, COUT], dt.bfloat16,
                           kind="ExternalInput")   # [Wlo; Wlo]
    wsh_d = nc.dram_tensor("wsh", [2 * CIN, COUT], dt.bfloat16,
                           kind="ExternalInput")
    wsl_d = nc.dram_tensor("wsl", [2 * CIN, COUT], dt.bfloat16,
                           kind="ExternalInput")
    w2t_d = nc.dram_tensor("w2t", [COUT, 9, COUT], dt.float16,
                           kind="ExternalInput")
    # columns: bn1_g, bn1_b, bn2_g, bn2_b, scn_g, scn_b
    pars_d = nc.dram_tensor("pars", [COUT, 6], dt.float32, kind="ExternalInput")
    out_d = nc.dram_tensor("out", [BPC, COUT, T, HO, WO], dt.uint8,
                           kind="ExternalOutput")

    from contextlib import ExitStack
    with tile.TileContext(nc) as tc, ExitStack() as stk:
        big = stk.enter_context(tc.tile_pool(name="big", bufs=1))
        const = stk.enter_context(tc.tile_pool(name="const", bufs=1))
        psum = stk.enter_context(tc.tile_pool(name="psum", bufs=6, space="PSUM"))
        dramp = stk.enter_context(tc.tile_pool(name="dramp", bufs=1, space="DRAM"))

        # ---- weights/params to SBUF (all contiguous) ----
        w1h = const.tile([2 * CIN, 9, COUT], dt.bfloat16)
        nc.sync.dma_start(out=w1h[:, :, :], in_=w1h_d[:, :, :])
        w1l = const.tile([2 * CIN, 9, COUT], dt.bfloat16)
        nc.sync.dma_start(out=w1l[:, :, :], in_=w1l_d[:, :, :])
        wsh = const.tile([2 * CIN, COUT], dt.bfloat16)
        nc.sync.dma_start(out=wsh[:, :], in_=wsh_d[:, :])
        wsl = const.tile([2 * CIN, COUT], dt.bfloat16)
        nc.sync.dma_start(out=wsl[:, :], in_=wsl_d[:, :])
        w2 = const.tile([COUT, 9, COUT], dt.float16)
        nc.sync.dma_start(out=w2[:, :, :], in_=w2t_d[:, :, :])
        pars = const.tile([COUT, 6], dt.float32)
        nc.sync.dma_start(out=pars[:, :], in_=pars_d[:, :])
        eps_t = const.tile([COUT, 1], dt.float32)
        nc.vector.memset(eps_t[:, :], EPS)

        # warmup AllReduce: aligns the 8 cores' launch skew here (overlapped
        # with phase A) so the first real AllReduce doesn't absorb it
        wrm = const.tile([1, 1], dt.float32)
        nc.vector.memset(wrm[:, :], 0.0)
        ccwi = dramp.tile([1, 1], dt.float32)
        ccwo = dramp.tile([1, 1], dt.float32, addr_space="Shared")
        nc.sync.dma_start(out=ccwi[:, :], in_=wrm[:, :])
        nc.gpsimd.collective_compute(
            "AllReduce", Alu.add, replica_groups=[list(range(NCORES))],
            ins=[ccwi[:, :].opt()], outs=[ccwo[:, :].opt()])

        # ---- persistent activation buffers, (t, s)-major free layout ----
        cv1f = big.tile([COUT, NLOC], dt.float32)   # conv1 raw, then c'=cv1+btil
        scf = big.tile([COUT, NLOC], dt.float32)    # shortcut raw, then thr
        cv2f = big.tile([COUT, NLOC], dt.float32)   # conv2 raw
        st1 = const.tile([COUT, 4 * T, 6], dt.float32)
        sts = const.tile([COUT, 4 * T, 6], dt.float32)
        st2 = const.tile([COUT, 4 * T, 6], dt.float32)

        # ===== phase A: conv1 + shortcut (bf16 hi/lo, partial-range) ========
        # conv1 tap (kh,kw): input (2oh+kh-1, 2ow+kw-1); kh=0 needs oh>=1,
        # kw=0 needs ow>=1, all else full. Chunk c covers oh in [14c, 14c+14).
        def tap_geom(kh, kw, c):
            if kh == 0:
                oh0 = 1 if c == 0 else 14
                nr = 13 if c == 0 else 14
            else:
                oh0, nr = 14 * c, 14
            rbase = (2 * oh0 + kh - 1) * W
            if kw == 0:
                cbase, ncol, oc = 1, WO - 1, 1
            else:
                cbase, ncol, oc = kw - 1, WO, 0
            orow = oh0 - 14 * c
            return rbase + cbase, orow * WO + oc, nr, ncol

        with tc.tile_pool(name="xq", bufs=3) as xpool:
            for t in range(T):
                for s in range(BPC):
                    xq = xpool.tile([2 * CIN, HW], dt.bfloat16, tag="xq")
                    nc.sync.dma_start(
                        out=_ap(xq[0:CIN, 0], 0, [[1, HW]]),
                        in_=xhi_d.ap()[s, :, t, :, :].rearrange("c h w -> c (h w)"))
                    nc.sync.dma_start(
                        out=_ap(xq[CIN:2 * CIN, 0], 0, [[1, HW]]),
                        in_=xlo_d.ap()[s, :, t, :, :].rearrange("c h w -> c (h w)"))
                    xb2 = xq[:, 0]
                    for c in range(2):
                        ps1 = psum.tile([COUT, CHUNK], dt.float32, tag="mm")
                        first = True
                        # center tap first: full range, starts PSUM
                        for k in (4, 0, 1, 2, 3, 5, 6, 7, 8):
                            kh, kw = divmod(k, 3)
                            ro, oo, nr, ncol = tap_geom(kh, kw, c)
                            rhs = _ap(xb2, ro, [[2 * W, nr], [2, ncol]])
                            outap = (ps1[:, :] if (nr == 14 and ncol == WO)
                                     else _ap(ps1[:, 0], oo,
                                              [[WO, nr], [1, ncol]]))
                            nc.tensor.matmul(outap, w1h[:, k, :], rhs,
                                             start=first, stop=False,
                                             skip_group_check=True)
                            nc.tensor.matmul(outap, w1l[:, k, :], rhs,
                                             start=False, stop=(k == 8),
                                             skip_group_check=True)
                            first = False
                        off = (t * BPC + s) * NPIX + c * CHUNK
                        idx = 4 * t + 2 * s + c
                        nc.scalar.copy(cv1f[:, off:off + CHUNK], ps1[:, :])
                        nc.vector.bn_stats(out=st1[:, idx, :], in_=ps1[:, :])
                        # shortcut 1x1 stride2: rows 2oh, cols 2ow (full range)
                        ps2 = psum.tile([COUT, CHUNK], dt.float32, tag="mm")
                        rhs = _ap(xb2, 28 * c * W, [[2 * W, 14], [2, WO]])
                        nc.tensor.matmul(ps2[:, :], wsh[:, :], rhs,
                                         start=True, stop=False,
                                         skip_group_check=True)
                        nc.tensor.matmul(ps2[:, :], wsl[:, :], rhs,
                                         start=False, stop=True,
                                         skip_group_check=True)
                        nc.scalar.copy(scf[:, off:off + CHUNK], ps2[:, :])
                        nc.vector.bn_stats(out=sts[:, idx, :], in_=ps2[:, :])

        # ---- local stats -> (sum, sumsq) -> AllReduce #1 ----
        mv1 = const.tile([COUT, 2], dt.float32)
        nc.vector.bn_aggr(out=mv1[:, :], in_=st1[:, :, :])
        mvs = const.tile([COUT, 2], dt.float32)
        nc.vector.bn_aggr(out=mvs[:, :], in_=sts[:, :, :])
        ar1 = const.tile([COUT, 4], dt.float32)
        for mv, base in ((mv1, 0), (mvs, 2)):
            nc.vector.tensor_scalar_mul(ar1[:, base:base + 1], mv[:, 0:1],
                                        float(NLOC))
            nc.vector.scalar_tensor_tensor(
                ar1[:, base + 1:base + 2], mv[:, 0:1], float(NLOC), mv[:, 0:1],
                Alu.mult, Alu.mult)
            nc.vector.scalar_tensor_tensor(
                ar1[:, base + 1:base + 2], mv[:, 1:2], float(NLOC),
                ar1[:, base + 1:base + 2], Alu.mult, Alu.add)
        cc1i = dramp.tile([COUT, 4], dt.float32)
        cc1o = dramp.tile([COUT, 4], dt.float32, addr_space="Shared")
        nc.sync.dma_start(out=cc1i[:, :], in_=ar1[:, :])
        nc.gpsimd.collective_compute(
            "AllReduce", Alu.add, replica_groups=[list(range(NCORES))],
            ins=[cc1i[:, :].opt()], outs=[cc1o[:, :].opt()])
        gs1 = const.tile([COUT, 4], dt.float32)
        nc.sync.dma_start(out=gs1[:, :], in_=cc1o[:, :])

        def mk_bn_consts(sums, g, b, tag):
            """global (sum,sumsq) [128,2] -> a = g*rstd, bb = b - a*mean."""
            mean = const.tile([COUT, 1], dt.float32, tag=tag + "_mean")
            nc.vector.tensor_scalar_mul(mean[:, :], sums[:, 0:1], 1.0 / NGLOB)
            var = const.tile([COUT, 1], dt.float32, tag=tag + "_var")
            nc.vector.tensor_scalar_mul(var[:, :], sums[:, 1:2], 1.0 / NGLOB)
            m2 = const.tile([COUT, 1], dt.float32, tag=tag + "_m2")
            nc.vector.tensor_tensor(m2[:, :], mean[:, :], mean[:, :], Alu.mult)
            nc.vector.tensor_tensor(var[:, :], var[:, :], m2[:, :], Alu.subtract)
            a = const.tile([COUT, 1], dt.float32, tag=tag + "_a")
            nc.scalar.activation(a[:, :], var[:, :], Act.Sqrt, bias=eps_t[:, :])
            nc.vector.reciprocal(a[:, :], a[:, :])
            nc.vector.tensor_tensor(a[:, :], a[:, :], g[:, :], Alu.mult)
            bb = const.tile([COUT, 1], dt.float32, tag=tag + "_bb")
            nc.vector.tensor_tensor(bb[:, :], a[:, :], mean[:, :], Alu.mult)
            nc.vector.tensor_tensor(bb[:, :], b[:, :], bb[:, :], Alu.subtract)
            return a, bb

        a1, b1 = mk_bn_consts(gs1[:, 0:2], pars[:, 0:1], pars[:, 1:2], "bn1")
        asc, bsc = mk_bn_consts(gs1[:, 2:4], pars[:, 4:5], pars[:, 5:6], "scn")

        # tau = 0.5/a1 ; btil = b1/a1  (a1 > 0 since gamma=1 at init)
        ra1 = const.tile([COUT, 1], dt.float32)
        nc.vector.reciprocal(ra1[:, :], a1[:, :])
        tau = const.tile([COUT, 1], dt.float32)
        nc.vector.tensor_scalar_mul(tau[:, :], ra1[:, :], 0.5)
        btil = const.tile([COUT, 1], dt.float32)
        nc.vector.tensor_tensor(btil[:, :], b1[:, :], ra1[:, :], Alu.mult)
        # shortcut threshold part 1: scf <- -asc*scf + (0.5 - bsc)
        nasc = const.tile([COUT, 1], dt.float32)
        nc.vector.tensor_scalar_mul(nasc[:, :], asc[:, :], -1.0)
        c1t = const.tile([COUT, 1], dt.float32)
        nc.vector.tensor_scalar(c1t[:, :], bsc[:, :], -1.0, 0.5, Alu.mult, Alu.add)

        # ============ phase B: LIF recurrence + conv2 (fp16, partial) =======
        def fold(t):  # c' = cv1 + btil, in place, one fused (s-pair) slice
            sl = cv1f[:, t * SPT:(t + 1) * SPT]
            nc.scalar.activation(sl, sl, Act.Identity, bias=btil[:, :])

        with tc.tile_pool(name="pu", bufs=2) as pu, \
             tc.tile_pool(name="pv", bufs=2) as pv, \
             tc.tile_pool(name="psp", bufs=3) as psp:

            def spike(t, v_ap):  # contiguous fp16 {0,1} tile, both samples
                # NB: must be on Vector: GpSimd's fp16 store path runs
                # ~16ns/elem and starves concurrent DVE SBUF access.
                sq = psp.tile([COUT, SPT], dt.float16, tag="sq")
                nc.vector.tensor_scalar(sq[:, :], v_ap, tau[:, :], None,
                                        Alu.is_gt)
                return sq

            fold(0)
            v_prev = cv1f[:, 0:SPT]
            sq = spike(0, v_prev)
            nq = 0  # scf rescale quarters interleaved on gpsimd
            for t in range(T):
                if t + 1 < T:
                    fold(t + 1)
                    u = pu.tile([COUT, SPT], dt.float32, tag="u")
                    nc.vector.scalar_tensor_tensor(
                        u[:, :], v_prev, tau[:, :], v_prev, Alu.is_le, Alu.mult)
                    v = pv.tile([COUT, SPT], dt.float32, tag="v")
                    nc.vector.scalar_tensor_tensor(
                        v[:, :], u[:, :], float(d),
                        cv1f[:, (t + 1) * SPT:(t + 2) * SPT], Alu.mult, Alu.add)
                    v_prev = v[:, :]
                    sq_next = spike(t + 1, v_prev)
                else:
                    sq_next = None
                if t >= 1 and nq < 4:  # overlap scf threshold part 1
                    q0 = nq * (NLOC // 4)
                    sl = scf[:, q0:q0 + NLOC // 4]
                    nc.gpsimd.tensor_scalar(sl, sl, nasc[:, :], c1t[:, :],
                                            Alu.mult, Alu.add)
                    nq += 1
                sqb = sq[:, 0]
                for s in range(BPC):
                    for c in range(2):
                        ps3 = psum.tile([COUT, CHUNK], dt.float32, tag="mm")
                        so = s * NPIX
                        oh0 = 14 * c
                        for ki, k in enumerate((4, 0, 1, 2, 3, 5, 6, 7, 8)):
                            kh, kw = divmod(k, 3)
                            r0 = oh0 + kh - 1
                            nr, o_r = 14, 0
                            if r0 < 0:          # kh=0, c=0
                                r0, nr, o_r = 0, 13, 1
                            elif r0 + 13 > 27:  # kh=2, c=1
                                nr = 13
                            if kw == 0:
                                cb, ncol, o_c = 0, WO - 1, 1
                            elif kw == 2:
                                cb, ncol, o_c = 1, WO - 1, 0
                            else:
                                cb, ncol, o_c = 0, WO, 0
                            outap = (ps3[:, :] if (nr == 14 and ncol == WO)
                                     else _ap(ps3[:, 0], o_r * WO + o_c,
                                              [[WO, nr], [1, ncol]]))
                            nc.tensor.matmul(
                                outap, w2[:, k, :],
                                _ap(sqb, so + r0 * WO + cb,
                                    [[WO, nr], [1, ncol]]),
                                start=(ki == 0), stop=(ki == 8),
                                skip_group_check=True)
                        off = (t * BPC + s) * NPIX + c * CHUNK
                        idx = 4 * t + 2 * s + c
                        nc.scalar.copy(cv2f[:, off:off + CHUNK], ps3[:, :])
                        nc.vector.bn_stats(out=st2[:, idx, :], in_=ps3[:, :])
                sq = sq_next

        # ---- AllReduce #2 (bn2 stats) ----
        mv2 = const.tile([COUT, 2], dt.float32)
        nc.vector.bn_aggr(out=mv2[:, :], in_=st2[:, :, :])
        ar2 = const.tile([COUT, 2], dt.float32)
        nc.vector.tensor_scalar_mul(ar2[:, 0:1], mv2[:, 0:1], float(NLOC))
        nc.vector.scalar_tensor_tensor(ar2[:, 1:2], mv2[:, 0:1], float(NLOC),
                                       mv2[:, 0:1], Alu.mult, Alu.mult)
        nc.vector.scalar_tensor_tensor(ar2[:, 1:2], mv2[:, 1:2], float(NLOC),
                                       ar2[:, 1:2], Alu.mult, Alu.add)
        cc2i = dramp.tile([COUT, 2], dt.float32)
        cc2o = dramp.tile([COUT, 2], dt.float32, addr_space="Shared")
        nc.sync.dma_start(out=cc2i[:, :], in_=ar2[:, :])
        nc.gpsimd.collective_compute(
            "AllReduce", Alu.add, replica_groups=[list(range(NCORES))],
            ins=[cc2i[:, :].opt()], outs=[cc2o[:, :].opt()])
        gs2 = const.tile([COUT, 2], dt.float32)
        nc.sync.dma_start(out=gs2[:, :], in_=cc2o[:, :])

        a2, b2 = mk_bn_consts(gs2[:, 0:2], pars[:, 2:3], pars[:, 3:4], "bn2")

        # out = 1[a2*cv2 + b2 > scf2]: z on the Scalar engine (per-partition
        # scale/bias APs), compare on Vector, pipelined per t with DMA out
        with tc.tile_pool(name="outp", bufs=2) as op, \
             tc.tile_pool(name="zp", bufs=2) as zp:
            for t in range(T):
                off = t * SPT
                z = zp.tile([COUT, SPT], dt.float32, tag="z")
                nc.scalar.activation(z[:, :], cv2f[:, off:off + SPT],
                                     Act.Identity, bias=b2[:, :],
                                     scale=a2[:, :])
                ot = op.tile([COUT, SPT], dt.uint8, tag="ot")
                nc.vector.tensor_tensor(ot[:, :], z[:, :],
                                        scf[:, off:off + SPT], Alu.is_gt)
                for s in range(BPC):
                    nc.sync.dma_start(
                        out=out_d.ap()[s, :, t, :, :].rearrange("c h w -> c (h w)"),
                        in_=ot[:, s * NPIX:(s + 1) * NPIX])

    nc.compile()
    return nc


_CACHE = {}


def _bf16_hilo(a):
    import ml_dtypes
    a = np.asarray(a, np.float32)
    hi = a.astype(ml_dtypes.bfloat16)
    lo = (a - hi.astype(np.float32)).astype(ml_dtypes.bfloat16)
    return hi, lo


def _host_prep(inputs):
    xhi, xlo = _bf16_hilo(inputs["x"])
    xhi, xlo = np.ascontiguousarray(xhi), np.ascontiguousarray(xlo)
    w1t = np.ascontiguousarray(inputs["cv1_w"], np.float32).reshape(
        COUT, CIN, 3, 3).transpose(1, 2, 3, 0).reshape(CIN, 9, COUT)
    w1hi, w1lo = _bf16_hilo(w1t)
    w1h = np.ascontiguousarray(np.concatenate([w1hi, w1hi], axis=0))
    w1l = np.ascontiguousarray(np.concatenate([w1lo, w1lo], axis=0))
    wst = np.asarray(inputs["sc_w"], np.float32).reshape(COUT, CIN).T
    wshi, wslo = _bf16_hilo(wst)
    wsh = np.ascontiguousarray(np.concatenate([wshi, wshi], axis=0))
    wsl = np.ascontiguousarray(np.concatenate([wslo, wslo], axis=0))
    w2t = np.ascontiguousarray(inputs["cv2_w"], np.float32).reshape(
        COUT, COUT, 3, 3).transpose(1, 2, 3, 0).reshape(COUT, 9, COUT)
    w2t = np.ascontiguousarray(w2t.astype(np.float16))
    pars = np.ascontiguousarray(np.stack(
        [np.asarray(inputs[p], np.float32).ravel()
         for p in ["bn1_g", "bn1_b", "bn2_g", "bn2_b", "scn_g", "scn_b"]],
        axis=1))
    d = float(1.0 / (1.0 + math.exp(-float(np.asarray(inputs["decay"]).ravel()[0]))))

    in_maps = []
    for c in range(NCORES):
        m = {"xhi": xhi[c * BPC:(c + 1) * BPC], "xlo": xlo[c * BPC:(c + 1) * BPC],
             "w1h": w1h, "w1l": w1l, "wsh": wsh, "wsl": wsl,
             "w2t": w2t, "pars": pars}
        in_maps.append(m)
    return in_maps, d


def kernel(**inputs):
    in_maps, d = _host_prep(inputs)
    key = round(d, 12)
    if key not in _CACHE:
        _CACHE[key] = build_nc(d)
    nc = _CACHE[key]

    res = run_bass_kernel_spmd(nc, in_maps, core_ids=list(range(NCORES)))
    out = np.concatenate([res.results[c]["out"] for c in range(NCORES)], axis=0)
    return np.ascontiguousarray(out, dtype=np.float32)


# revision 13
# speedup vs baseline: 1.3105x; 1.3105x over previous
"""LIAFResBlock forward on 8 Trainium2 NeuronCores (data-parallel over batch).

Self-contained: hardcodes shapes for x [16,64,8,56,56] -> out [16,128,8,28,28].

Math notes (vs the PyTorch/JAX reference):
  - conv biases are no-ops: every conv is followed by training-mode BN, which
    subtracts the per-channel mean, absorbing any per-channel constant.
  - the final mem_update on a binary {0,1} tensor is the identity because
    d = sigmoid(0.5) ~ 0.6225 and d*0.5 < 0.5, so out = lif_act(bn2(cv2)+bn_sc(sc)).
  - the first mem_update runs in "normalized" space: with a1 = g1*rstd1 (>0),
    v = m/a1 satisfies v[t] = d*v[t-1]*[v<=tau] + (cv1[t] + beta1/a1),
    spike[t] = v[t] > tau, tau = 0.5/a1. BN1 folds into a per-channel bias on
    cv1 plus a per-channel threshold.
  - BN batch stats are global over B=16: each core computes per-channel
    (sum, sumsq) partials; a tiny AllReduce combines them.
  - final threshold in raw-cv2 space: out = 1[cv2_raw > (0.5 - bsc - asc*sc
    - b2)/a2] (a2 > 0 at init since gamma2 = 1).

Implementation notes (hardware-measured):
  - conv1/shortcut run as bf16 hi+lo pairs: x = xhi + xlo and W = Whi + Wlo
    (each bf16, split host-side), rhs K=128 stacks [xhi; xlo] and each tap
    issues two 1-cycle/row streams, lhsT [Whi;Whi] then [Wlo;Wlo], giving
    (Whi+Wlo)*(xhi+xlo): bf16 products are exact in the fp32 PSUM, so this is
    an fp32 conv of ~2^-17-rounded operands (emulated: 749 output flips, rel
    1.27e-2, vs budget 2e-2). A plain fp32 conv1 costs ~2x more PE time;
    any tf32/bf16-level rounding of x explodes ~30x through the LIF
    recurrence (16k flips).
  - conv2 fp16 single-pass (spikes are exact {0,1} in fp16; w2 fp16 rounding
    costs ~590 flips).
  - zero padding is done with partial-range matmuls (each off-center tap
    writes only its valid output sub-rectangle of PSUM; the center tap goes
    first with start=True covering the full chunk), NOT with zero-ringed
    input tiles: padded tiles force 224-byte DMA descriptors (~96 GB/s
    effective, 262us for x) while unpadded tiles stream at 6.3KB/descriptor.
  - spikes are written as contiguous [128, 1568] fp16: strided elementwise
    writes on Vector/Pool run ~18ns/elem AND starve the other engine's SBUF
    access (measured 28us stretches on concurrently-running DVE ops).
  - weights are transposed host-side so every DMA is contiguous (on-device
    transposing DMA = 230k 4-byte packets = 170us).
"""
import math
import sys

import numpy as np

sys.path.insert(0, "/opt/trn_rl_repo")

import concourse.bass as bass  # noqa: E402
import concourse.bacc as bacc  # noqa: E402
import concourse.tile as tile  # noqa: E402
from concourse import mybir  # noqa: E402
from concourse.bass_utils import run_bass_kernel_spmd  # noqa: E402

dt = mybir.dt
Alu = mybir.AluOpType
Act = mybir.ActivationFunctionType

B, CIN, COUT, T, H, W = 16, 64, 128, 8, 56, 56
HO = WO = 28
NPIX = HO * WO          # 784
CHUNK = NPIX // 2       # 392 (one PSUM bank)
NCORES = 8
BPC = B // NCORES       # 2 samples per core
SPT = BPC * NPIX        # 1568 elements per fused (both-samples) timestep
NLOC = BPC * T * NPIX   # 12544 elements/channel per core
NGLOB = B * T * NPIX    # 100352 elements/channel globally
EPS = 1e-5
HW = H * W              # 3136 elements per unpadded input plane


def _ap(base, off, free):
    """Sub-view of an SBUF AP: keep partition dim, custom free dims."""
    return bass.AP(tensor=base.tensor, offset=base.offset + off,
                   ap=[base.ap[0]] + free)


def build_nc(d: float) -> bass.Bass:
    nc = bacc.Bacc("TRN2", target_bir_lowering=False, num_devices=NCORES)

    xhi_d = nc.dram_tensor("xhi", [BPC, CIN, T, H, W], dt.bfloat16,
                           kind="ExternalInput")
    xlo_d = nc.dram_tensor("xlo", [BPC, CIN, T, H, W], dt.bfloat16,
                           kind="ExternalInput")
    # host-pretransposed lhsT weights, hi/lo split, duplicated across K halves
    w1h_d = nc.dram_tensor("w1h", [2 * CIN, 9, COUT], dt.bfloat16,
                           kind="ExternalInput")   # [Whi; Whi]
    w1l_d = nc.dram_tensor("w1l", [2 * CIN, 9, COUT], dt.bfloat16,
                           kind="ExternalInput")   # [Wlo; Wlo]
    wsh_d = nc.dram_tensor("wsh", [2 * CIN, COUT], dt.bfloat16,
                           kind="ExternalInput")
    wsl_d = nc.dram_tensor("wsl", [2 * CIN, COUT], dt.bfloat16,
                           kind="ExternalInput")
    w2t_d = nc.dram_tensor("w2t", [COUT, 9, COUT], dt.float16,
                           kind="ExternalInput")
    # columns: bn1_g, bn1_b, bn2_g, bn2_b, scn_g, scn_b
    pars_d = nc.dram_tensor("pars", [COUT, 6], dt.float32, kind="ExternalInput")
    out_d = nc.dram_tensor("out", [BPC, COUT, T, HO, WO], dt.uint8,
                           kind="ExternalOutput")

    from contextlib import ExitStack
    with tile.TileContext(nc) as tc, ExitStack() as stk:
        big = stk.enter_context(tc.tile_pool(name="big", bufs=1))
        const = stk.enter_context(tc.tile_pool(name="const", bufs=1))
        psum = stk.enter_context(tc.tile_pool(name="psum", bufs=8, space="PSUM"))
        dramp = stk.enter_context(tc.tile_pool(name="dramp", bufs=1, space="DRAM"))

        # ---- weights/params to SBUF (all contiguous) ----
        w1h = const.tile([2 * CIN, 9, COUT], dt.bfloat16)
        nc.sync.dma_start(out=w1h[:, :, :], in_=w1h_d[:, :, :])
        w1l = const.tile([2 * CIN, 9, COUT], dt.bfloat16)
        nc.sync.dma_start(out=w1l[:, :, :], in_=w1l_d[:, :, :])
        wsh = const.tile([2 * CIN, COUT], dt.bfloat16)
        nc.sync.dma_start(out=wsh[:, :], in_=wsh_d[:, :])
        wsl = const.tile([2 * CIN, COUT], dt.bfloat16)
        nc.sync.dma_start(out=wsl[:, :], in_=wsl_d[:, :])
        w2 = const.tile([COUT, 9, COUT], dt.float16)
        nc.sync.dma_start(out=w2[:, :, :], in_=w2t_d[:, :, :])
        pars = const.tile([COUT, 6], dt.float32)
        nc.sync.dma_start(out=pars[:, :], in_=pars_d[:, :])
        eps_t = const.tile([COUT, 1], dt.float32)
        nc.vector.memset(eps_t[:, :], EPS)

        # warmup AllReduce: aligns the 8 cores' launch skew here (overlapped
        # with phase A) so the first real AllReduce doesn't absorb it
        wrm = const.tile([1, 1], dt.float32)
        nc.vector.memset(wrm[:, :], 0.0)
        ccwi = dramp.tile([1, 1], dt.float32)
        ccwo = dramp.tile([1, 1], dt.float32, addr_space="Shared")
        nc.sync.dma_start(out=ccwi[:, :], in_=wrm[:, :])
        nc.gpsimd.collective_compute(
            "AllReduce", Alu.add, replica_groups=[list(range(NCORES))],
            ins=[ccwi[:, :].opt()], outs=[ccwo[:, :].opt()])

        # ---- persistent activation buffers, (t, s)-major free layout ----
        cv1f = big.tile([COUT, NLOC], dt.float32)   # conv1 raw, then c'=cv1+btil
        scf = big.tile([COUT, NLOC], dt.float32)    # shortcut raw, then thr
        cv2f = big.tile([COUT, NLOC], dt.float32)   # conv2 raw
        st1 = const.tile([COUT, 4 * T, 6], dt.float32)
        sts = const.tile([COUT, 4 * T, 6], dt.float32)
        st2 = const.tile([COUT, 4 * T, 6], dt.float32)

        # ===== phase A: conv1 + shortcut (bf16 hi/lo, partial-range) ========
        # conv1 tap (kh,kw): input (2oh+kh-1, 2ow+kw-1); kh=0 needs oh>=1,
        # kw=0 needs ow>=1, all else full. Chunk c covers oh in [14c, 14c+14).
        def tap_geom(kh, kw, c):
            if kh == 0:
                oh0 = 1 if c == 0 else 14
                nr = 13 if c == 0 else 14
            else:
                oh0, nr = 14 * c, 14
            rbase = (2 * oh0 + kh - 1) * W
            if kw == 0:
                cbase, ncol, oc = 1, WO - 1, 1
            else:
                cbase, ncol, oc = kw - 1, WO, 0
            orow = oh0 - 14 * c
            return rbase + cbase, orow * WO + oc, nr, ncol

        with tc.tile_pool(name="xq", bufs=4) as xpool:
            for t in range(T):
                for s in range(BPC):
                    xq = xpool.tile([2 * CIN, HW], dt.bfloat16, tag="xq")
                    nc.sync.dma_start(
                        out=_ap(xq[0:CIN, 0], 0, [[1, HW]]),
                        in_=xhi_d.ap()[s, :, t, :, :].rearrange("c h w -> c (h w)"))
                    nc.sync.dma_start(
                        out=_ap(xq[CIN:2 * CIN, 0], 0, [[1, HW]]),
                        in_=xlo_d.ap()[s, :, t, :, :].rearrange("c h w -> c (h w)"))
                    xb2 = xq[:, 0]
                    for c in range(2):
                        ps1 = psum.tile([COUT, CHUNK], dt.float32, tag="mm")
                        first = True
                        # center tap first: full range, starts PSUM
                        for k in (4, 0, 1, 2, 3, 5, 6, 7, 8):
                            kh, kw = divmod(k, 3)
                            ro, oo, nr, ncol = tap_geom(kh, kw, c)
                            rhs = _ap(xb2, ro, [[2 * W, nr], [2, ncol]])
                            outap = (ps1[:, :] if (nr == 14 and ncol == WO)
                                     else _ap(ps1[:, 0], oo,
                                              [[WO, nr], [1, ncol]]))
                            nc.tensor.matmul(outap, w1h[:, k, :], rhs,
                                             start=first, stop=False,
                                             skip_group_check=True)
                            nc.tensor.matmul(outap, w1l[:, k, :], rhs,
                                             start=False, stop=(k == 8),
                                             skip_group_check=True)
                            first = False
                        off = (t * BPC + s) * NPIX + c * CHUNK
                        idx = 4 * t + 2 * s + c
                        nc.scalar.copy(cv1f[:, off:off + CHUNK], ps1[:, :])
                        nc.vector.bn_stats(out=st1[:, idx, :], in_=ps1[:, :])
                        # shortcut 1x1 stride2: rows 2oh, cols 2ow (full range)
                        ps2 = psum.tile([COUT, CHUNK], dt.float32, tag="mm")
                        rhs = _ap(xb2, 28 * c * W, [[2 * W, 14], [2, WO]])
                        nc.tensor.matmul(ps2[:, :], wsh[:, :], rhs,
                                         start=True, stop=False,
                                         skip_group_check=True)
                        nc.tensor.matmul(ps2[:, :], wsl[:, :], rhs,
                                         start=False, stop=True,
                                         skip_group_check=True)
                        nc.scalar.copy(scf[:, off:off + CHUNK], ps2[:, :])
                        nc.vector.bn_stats(out=sts[:, idx, :], in_=ps2[:, :])

        # ---- local stats -> (sum, sumsq) -> AllReduce #1 ----
        mv1 = const.tile([COUT, 2], dt.float32)
        nc.vector.bn_aggr(out=mv1[:, :], in_=st1[:, :, :])
        mvs = const.tile([COUT, 2], dt.float32)
        nc.vector.bn_aggr(out=mvs[:, :], in_=sts[:, :, :])
        ar1 = const.tile([COUT, 4], dt.float32)
        for mv, base in ((mv1, 0), (mvs, 2)):
            nc.vector.tensor_scalar_mul(ar1[:, base:base + 1], mv[:, 0:1],
                                        float(NLOC))
            nc.vector.scalar_tensor_tensor(
                ar1[:, base + 1:base + 2], mv[:, 0:1], float(NLOC), mv[:, 0:1],
                Alu.mult, Alu.mult)
            nc.vector.scalar_tensor_tensor(
                ar1[:, base + 1:base + 2], mv[:, 1:2], float(NLOC),
                ar1[:, base + 1:base + 2], Alu.mult, Alu.add)
        cc1i = dramp.tile([COUT, 4], dt.float32)
        cc1o = dramp.tile([COUT, 4], dt.float32, addr_space="Shared")
        nc.sync.dma_start(out=cc1i[:, :], in_=ar1[:, :])
        nc.gpsimd.collective_compute(
            "AllReduce", Alu.add, replica_groups=[list(range(NCORES))],
            ins=[cc1i[:, :].opt()], outs=[cc1o[:, :].opt()])
        gs1 = const.tile([COUT, 4], dt.float32)
        nc.sync.dma_start(out=gs1[:, :], in_=cc1o[:, :])

        def mk_bn_consts(sums, g, b, tag):
            """global (sum,sumsq) [128,2] -> a = g*rstd, bb = b - a*mean."""
            mean = const.tile([COUT, 1], dt.float32, tag=tag + "_mean")
            nc.vector.tensor_scalar_mul(mean[:, :], sums[:, 0:1], 1.0 / NGLOB)
            var = const.tile([COUT, 1], dt.float32, tag=tag + "_var")
            nc.vector.tensor_scalar_mul(var[:, :], sums[:, 1:2], 1.0 / NGLOB)
            m2 = const.tile([COUT, 1], dt.float32, tag=tag + "_m2")
            nc.vector.tensor_tensor(m2[:, :], mean[:, :], mean[:, :], Alu.mult)
            nc.vector.tensor_tensor(var[:, :], var[:, :], m2[:, :], Alu.subtract)
            a = const.tile([COUT, 1], dt.float32, tag=tag + "_a")
            nc.scalar.activation(a[:, :], var[:, :], Act.Sqrt, bias=eps_t[:, :])
            nc.vector.reciprocal(a[:, :], a[:, :])
            nc.vector.tensor_tensor(a[:, :], a[:, :], g[:, :], Alu.mult)
            bb = const.tile([COUT, 1], dt.float32, tag=tag + "_bb")
            nc.vector.tensor_tensor(bb[:, :], a[:, :], mean[:, :], Alu.mult)
            nc.vector.tensor_tensor(bb[:, :], b[:, :], bb[:, :], Alu.subtract)
            return a, bb

        a1, b1 = mk_bn_consts(gs1[:, 0:2], pars[:, 0:1], pars[:, 1:2], "bn1")
        asc, bsc = mk_bn_consts(gs1[:, 2:4], pars[:, 4:5], pars[:, 5:6], "scn")

        # tau = 0.5/a1 ; btil = b1/a1  (a1 > 0 since gamma=1 at init)
        ra1 = const.tile([COUT, 1], dt.float32)
        nc.vector.reciprocal(ra1[:, :], a1[:, :])
        tau = const.tile([COUT, 1], dt.float32)
        nc.vector.tensor_scalar_mul(tau[:, :], ra1[:, :], 0.5)
        btil = const.tile([COUT, 1], dt.float32)
        nc.vector.tensor_tensor(btil[:, :], b1[:, :], ra1[:, :], Alu.mult)
        # shortcut threshold part 1: scf <- -asc*scf + (0.5 - bsc)
        nasc = const.tile([COUT, 1], dt.float32)
        nc.vector.tensor_scalar_mul(nasc[:, :], asc[:, :], -1.0)
        c1t = const.tile([COUT, 1], dt.float32)
        nc.vector.tensor_scalar(c1t[:, :], bsc[:, :], -1.0, 0.5, Alu.mult, Alu.add)

        # ============ phase B: LIF recurrence + conv2 (fp16, partial) =======
        def fold(t):  # c' = cv1 + btil, in place, one fused (s-pair) slice
            sl = cv1f[:, t * SPT:(t + 1) * SPT]
            nc.scalar.activation(sl, sl, Act.Identity, bias=btil[:, :])

        with tc.tile_pool(name="pu", bufs=2) as pu, \
             tc.tile_pool(name="pv", bufs=2) as pv, \
             tc.tile_pool(name="psp", bufs=3) as psp:

            def spike(t, v_ap):  # contiguous fp16 {0,1} tile, both samples
                # NB: must be on Vector: GpSimd's fp16 store path runs
                # ~16ns/elem and starves concurrent DVE SBUF access.
                sq = psp.tile([COUT, SPT], dt.float16, tag="sq")
                nc.vector.tensor_scalar(sq[:, :], v_ap, tau[:, :], None,
                                        Alu.is_gt)
                return sq

            fold(0)
            v_prev = cv1f[:, 0:SPT]
            sq = spike(0, v_prev)
            nq = 0  # scf rescale quarters interleaved on gpsimd
            for t in range(T):
                if t + 1 < T:
                    fold(t + 1)
                    u = pu.tile([COUT, SPT], dt.float32, tag="u")
                    nc.vector.scalar_tensor_tensor(
                        u[:, :], v_prev, tau[:, :], v_prev, Alu.is_le, Alu.mult)
                    v = pv.tile([COUT, SPT], dt.float32, tag="v")
                    nc.vector.scalar_tensor_tensor(
                        v[:, :], u[:, :], float(d),
                        cv1f[:, (t + 1) * SPT:(t + 2) * SPT], Alu.mult, Alu.add)
                    v_prev = v[:, :]
                    sq_next = spike(t + 1, v_prev)
                else:
                    sq_next = None
                if t >= 1 and nq < 4:  # overlap scf threshold part 1
                    q0 = nq * (NLOC // 4)
                    sl = scf[:, q0:q0 + NLOC // 4]
                    nc.gpsimd.tensor_scalar(sl, sl, nasc[:, :], c1t[:, :],
                                            Alu.mult, Alu.add)
                    nq += 1
                sqb = sq[:, 0]
                for s in range(BPC):
                    for c in range(2):
                        ps3 = psum.tile([COUT, CHUNK], dt.float32, tag="mm")
                        so = s * NPIX
                        oh0 = 14 * c
                        for ki, k in enumerate((4, 0, 1, 2, 3, 5, 6, 7, 8)):
                            kh, kw = divmod(k, 3)
                            r0 = oh0 + kh - 1
                            nr, o_r = 14, 0
                            if r0 < 0:          # kh=0, c=0
                                r0, nr, o_r = 0, 13, 1
                            elif r0 + 13 > 27:  # kh=2, c=1
                                nr = 13
                            if kw == 0:
                                cb, ncol, o_c = 0, WO - 1, 1
                            elif kw == 2:
                                cb, ncol, o_c = 1, WO - 1, 0
                            else:
                                cb, ncol, o_c = 0, WO, 0
                            outap = (ps3[:, :] if (nr == 14 and ncol == WO)
                                     else _ap(ps3[:, 0], o_r * WO + o_c,
                                              [[WO, nr], [1, ncol]]))
                            nc.tensor.matmul(
                                outap, w2[:, k, :],
                                _ap(sqb, so + r0 * WO + cb,
                                    [[WO, nr], [1, ncol]]),
                                start=(ki == 0), stop=(ki == 8),
                                skip_group_check=True)
                        off = (t * BPC + s) * NPIX + c * CHUNK
                        idx = 4 * t + 2 * s + c
                        nc.scalar.copy(cv2f[:, off:off + CHUNK], ps3[:, :])
                        nc.vector.bn_stats(out=st2[:, idx, :], in_=ps3[:, :])
                sq = sq_next

        # ---- AllReduce #2 (bn2 stats) ----
        mv2 = const.tile([COUT, 2], dt.float32)
        nc.vector.bn_aggr(out=mv2[:, :], in_=st2[:, :, :])
        ar2 = const.tile([COUT, 2], dt.float32)
        nc.vector.tensor_scalar_mul(ar2[:, 0:1], mv2[:, 0:1], float(NLOC))
        nc.vector.scalar_tensor_tensor(ar2[:, 1:2], mv2[:, 0:1], float(NLOC),
                                       mv2[:, 0:1], Alu.mult, Alu.mult)
        nc.vector.scalar_tensor_tensor(ar2[:, 1:2], mv2[:, 1:2], float(NLOC),
                                       ar2[:, 1:2], Alu.mult, Alu.add)
        cc2i = dramp.tile([COUT, 2], dt.float32)
        cc2o = dramp.tile([COUT, 2], dt.float32, addr_space="Shared")
        nc.sync.dma_start(out=cc2i[:, :], in_=ar2[:, :])
        nc.gpsimd.collective_compute(
            "AllReduce", Alu.add, replica_groups=[list(range(NCORES))],
            ins=[cc2i[:, :].opt()], outs=[cc2o[:, :].opt()])
        gs2 = const.tile([COUT, 2], dt.float32)
        nc.sync.dma_start(out=gs2[:, :], in_=cc2o[:, :])

        a2, b2 = mk_bn_consts(gs2[:, 0:2], pars[:, 2:3], pars[:, 3:4], "bn2")

        # out = 1[a2*cv2 + b2 > scf2]: z on the Scalar engine (per-partition
        # scale/bias APs), compare on Vector, pipelined per t with DMA out
        with tc.tile_pool(name="outp", bufs=2) as op, \
             tc.tile_pool(name="zp", bufs=2) as zp:
            for t in range(T):
                off = t * SPT
                z = zp.tile([COUT, SPT], dt.float32, tag="z")
                nc.scalar.activation(z[:, :], cv2f[:, off:off + SPT],
                                     Act.Identity, bias=b2[:, :],
                                     scale=a2[:, :])
                ot = op.tile([COUT, SPT], dt.uint8, tag="ot")
                nc.vector.tensor_tensor(ot[:, :], z[:, :],
                                        scf[:, off:off + SPT], Alu.is_gt)
                for s in range(BPC):
                    nc.sync.dma_start(
                        out=out_d.ap()[s, :, t, :, :].rearrange("c h w -> c (h w)"),
                        in_=ot[:, s * NPIX:(s + 1) * NPIX])

    nc.compile()
    return nc


_CACHE = {}


def _bf16_hilo(a):
    import ml_dtypes
    a = np.asarray(a, np.float32)
    hi = a.astype(ml_dtypes.bfloat16)
    lo = (a - hi.astype(np.float32)).astype(ml_dtypes.bfloat16)
    return hi, lo


def _host_prep(inputs):
    xhi, xlo = _bf16_hilo(inputs["x"])
    xhi, xlo = np.ascontiguousarray(xhi), np.ascontiguousarray(xlo)
    w1t = np.ascontiguousarray(inputs["cv1_w"], np.float32).reshape(
        COUT, CIN, 3, 3).transpose(1, 2, 3, 0).reshape(CIN, 9, COUT)
    w1hi, w1lo = _bf16_hilo(w1t)
    w1h = np.ascontiguousarray(np.concatenate([w1hi, w1hi], axis=0))
    w1l = np.ascontiguousarray(np.concatenate([w1lo, w1lo], axis=0))
    wst = np.asarray(inputs["sc_w"], np.float32).reshape(COUT, CIN).T
    wshi, wslo = _bf16_hilo(wst)
    wsh = np.ascontiguousarray(np.concatenate([wshi, wshi], axis=0))
    wsl = np.ascontiguousarray(np.concatenate([wslo, wslo], axis=0))
    w2t = np.ascontiguousarray(inputs["cv2_w"], np.float32).reshape(
        COUT, COUT, 3, 3).transpose(1, 2, 3, 0).reshape(COUT, 9, COUT)
    w2t = np.ascontiguousarray(w2t.astype(np.float16))
    pars = np.ascontiguousarray(np.stack(
        [np.asarray(inputs[p], np.float32).ravel()
         for p in ["bn1_g", "bn1_b", "bn2_g", "bn2_b", "scn_g", "scn_b"]],
        axis=1))
    d = float(1.0 / (1.0 + math.exp(-float(np.asarray(inputs["decay"]).ravel()[0]))))

    in_maps = []
    for c in range(NCORES):
        m = {"xhi": xhi[c * BPC:(c + 1) * BPC], "xlo": xlo[c * BPC:(c + 1) * BPC],
             "w1h": w1h, "w1l": w1l, "wsh": wsh, "wsl": wsl,
             "w2t": w2t, "pars": pars}
        in_maps.append(m)
    return in_maps, d


def kernel(**inputs):
    in_maps, d = _host_prep(inputs)
    key = round(d, 12)
    if key not in _CACHE:
        _CACHE[key] = build_nc(d)
    nc = _CACHE[key]

    res = run_bass_kernel_spmd(nc, in_maps, core_ids=list(range(NCORES)))
    out = np.concatenate([res.results[c]["out"] for c in range(NCORES)], axis=0)
    return np.ascontiguousarray(out, dtype=np.float32)
